# revision 4
# baseline (speedup 1.0000x reference)
"""Self-contained Trainium2 Bass kernel for nn_MultiHeadAttention_50053548868010.

Sharding (8 cores): core c handles batch b = c//2 and head-half g = c%2
(heads 8g..8g+7). Attention is fully head-parallel; the output projection
partial sums are combined with a pairwise ReduceScatter (cores 2k,2k+1),
each core then layer-norms its half of the rows.
"""

import sys

sys.path.insert(0, "/opt/trn_rl_repo")

import numpy as np

import concourse.bass as bass
import concourse.mybir as mybir
import concourse.tile as tile
from concourse import bacc
from concourse.bass_utils import run_bass_kernel_spmd
from concourse.masks import make_identity, make_causal_mask

# Problem dims (hardcoded per contest contract)
B, L, DX, H, DK, DV = 4, 2048, 1024, 16, 64, 64
EPS = 1e-5
N_CORES = 8
HL = H // 2          # 8 local heads per core
DHL = HL * DK        # 512 local head-dims
P = 128
XB = DX // P         # 8 x-subtiles
DS = DHL // P        # 4 d-subtiles (head pairs)
LB = L // P          # 16 l-blocks of 128
LC = L // 512        # 4 l-chunks of 512
HALF = L // 2

F32 = mybir.dt.float32
F32R = mybir.dt.float32r
AF = mybir.ActivationFunctionType
OP = mybir.AluOpType
RG = [[0, 1], [2, 3], [4, 5], [6, 7]]  # reduce-scatter pairs


def build(nc):
    # ---- DRAM I/O (per-core) ----
    q_in = nc.dram_tensor("q_in", [L, DX], F32R, kind="ExternalInput")
    k_in = nc.dram_tensor("k_in", [L, DX], F32R, kind="ExternalInput")
    v_in = nc.dram_tensor("v_in", [L, DX], F32R, kind="ExternalInput")
    wq = nc.dram_tensor("wq", [DX, DHL], F32R, kind="ExternalInput")
    wk = nc.dram_tensor("wk", [DX, DHL], F32R, kind="ExternalInput")
    wv = nc.dram_tensor("wv", [DX, DHL], F32R, kind="ExternalInput")
    wo = nc.dram_tensor("wo", [DHL, DX], F32R, kind="ExternalInput")
    bq_d = nc.dram_tensor("bq", [DHL], F32, kind="ExternalInput")
    bk_d = nc.dram_tensor("bk", [DHL], F32, kind="ExternalInput")
    bv_d = nc.dram_tensor("bv", [DHL], F32, kind="ExternalInput")
    bo_d = nc.dram_tensor("bo", [DX], F32, kind="ExternalInput")
    gam_d = nc.dram_tensor("gamma", [DX], F32, kind="ExternalInput")
    bet_d = nc.dram_tensor("beta", [DX], F32, kind="ExternalInput")

    att_out = nc.dram_tensor("att_out", [HL, L, L], F32, kind="ExternalOutput")
    y_out = nc.dram_tensor("y_out", [HALF, DX], F32, kind="ExternalOutput")

    with tile.TileContext(nc) as tc:
        with (
            tc.tile_pool(name="consts", bufs=1) as consts,
            tc.tile_pool(name="persist", bufs=1) as persist,
            tc.tile_pool(name="dram", bufs=1, space="DRAM") as drampool,
        ):
            identf = consts.tile([P, P], F32)
            make_identity(nc, identf[:])
            ident = consts.tile([P, P], F32R)
            nc.vector.tensor_copy(ident[:], identf[:])
            tri = consts.tile([P, P], F32)
            make_causal_mask(nc, tri[:], mask_val=-1e30)

            # biases along partition (d) for q/k copybacks
            bq_sb = consts.tile([P, DS], F32)
            nc.sync.dma_start(out=bq_sb[:], in_=bq_d.ap().rearrange("(ds p) -> p ds", p=P))
            bk_sb = consts.tile([P, DS], F32)
            nc.sync.dma_start(out=bk_sb[:], in_=bk_d.ap().rearrange("(ds p) -> p ds", p=P))
            # bv broadcast along partitions (hd on free dim)
            bv_bc = consts.tile([P, DHL], F32)
            nc.sync.dma_start(
                out=bv_bc[:],
                in_=bass.AP(tensor=bv_d.ap().tensor, offset=0, ap=[[0, P], [1, DHL]]),
            )

            # persistent activations
            qT = persist.tile([P, DS, L], F32R, name="qT")    # [d_in_pair, head-pair, l]
            kT = persist.tile([P, DS, L], F32R, name="kT")
            vS = persist.tile([P, LB, DHL], F32R, name="vS")  # [j_in_block, j_block, hd]
            attOT = persist.tile([P, DS, L], F32R, name="attOT")  # [hd_in_pair, pair, l]

            # ---- Phase 1+2: transpose inputs & project ----
            with (
                tc.tile_pool(name="wpool", bufs=1) as wpool,
                tc.tile_pool(name="natp", bufs=1) as natp,
                tc.tile_pool(name="ttp", bufs=1) as ttp,
                tc.tile_pool(name="ps_tp", bufs=2, space="PSUM") as ps_tp,
                tc.tile_pool(name="ps_pr", bufs=3, space="PSUM") as ps_pr,
            ):
                for which, (nat_d, w_d) in enumerate(
                    [(q_in, wq), (k_in, wk), (v_in, wv)]
                ):
                    w_sb = wpool.tile([P, XB, DHL], F32R, tag="w")
                    nc.sync.dma_start(
                        out=w_sb[:], in_=w_d.ap().rearrange("(xo p) d -> p xo d", p=P)
                    )
                    for lc in range(LC):
                        nat = natp.tile([P, 4, DX], F32R, tag="nat")
                        nc.sync.dma_start(
                            out=nat[:],
                            in_=nat_d.ap().rearrange("(lc lt p) x -> lc p lt x", p=P, lt=4)[lc],
                        )
                        tt = ttp.tile([P, XB, 512], F32R, tag="tt")
                        for xb in range(XB):
                            pt = ps_tp.tile([P, 512], F32R, tag="pstp")
                            for lt in range(4):
                                nc.tensor.transpose(
                                    pt[:, lt * P : (lt + 1) * P],
                                    nat[:, lt, xb * P : (xb + 1) * P],
                                    ident[:],
                                )
                            nc.vector.tensor_copy(tt[:, xb, :], pt[:])
                        if which < 2:
                            # q/k: output transposed [d, l]
                            for ds in range(DS):
                                pp = ps_pr.tile([P, 512], F32, tag="pspr")
                                for xb in range(XB):
                                    nc.tensor.matmul(
                                        pp[:],
                                        w_sb[:, xb, ds * P : (ds + 1) * P],
                                        tt[:, xb, :],
                                        start=(xb == 0),
                                        stop=(xb == XB - 1),
                                    )
                                dst = (qT if which == 0 else kT)[
                                    :, ds, lc * 512 : (lc + 1) * 512
                                ]
                                if which == 0:
                                    # (q + bq) * (1/sqrt(DK))
                                    nc.vector.tensor_scalar(
                                        dst, pp[:], bq_sb[:, ds : ds + 1],
                                        1.0 / float(np.sqrt(DK)),
                                        op0=OP.add, op1=OP.mult,
                                    )
                                else:
                                    nc.vector.tensor_scalar(
                                        dst, pp[:], bk_sb[:, ds : ds + 1], None,
                                        op0=OP.add,
                                    )
                        else:
                            # v: natural [l, hd]
                            for lt in range(4):
                                pp = ps_pr.tile([P, DHL], F32, tag="pspr")
                                for xb in range(XB):
                                    nc.tensor.matmul(
                                        pp[:],
                                        tt[:, xb, lt * P : (lt + 1) * P],
                                        w_sb[:, xb, :],
                                        start=(xb == 0),
                                        stop=(xb == XB - 1),
                                    )
                                nc.vector.tensor_tensor(
                                    vS[:, lc * 4 + lt, :], pp[:], bv_bc[:], OP.add
                                )

            # ---- Phase 3: attention per head ----
            with (
                tc.tile_pool(name="upool", bufs=6) as upool,
                tc.tile_pool(name="apool", bufs=5) as apool,
                tc.tile_pool(name="stage", bufs=17) as stagep,
                tc.tile_pool(name="rpool", bufs=4) as rpool,
                tc.tile_pool(name="ps_s", bufs=3, space="PSUM") as ps_s,
                tc.tile_pool(name="ps_t", bufs=2, space="PSUM") as ps_t,
                tc.tile_pool(name="ps_o", bufs=2, space="PSUM") as ps_o,
            ):
                for h in range(HL):
                    hp, hs = h // 2, (h % 2) * DK  # pair index, partition offset
                    stages = {}
                    for ib in range(LB):
                        ic = ib // 4
                        nch = ic + 1
                        u_chunks = []
                        racc = rpool.tile([P, 4], F32, tag="racc")
                        for jc in range(nch):
                            w = 512 if jc < ic else (ib % 4) * P + P
                            pssc = ps_s.tile([P, 512], F32, tag="score")
                            nc.tensor.matmul(
                                pssc[:],
                                qT[hs : hs + DK, hp, ib * P : (ib + 1) * P],
                                kT[hs : hs + DK, hp, jc * 512 : (jc + 1) * 512],
                                start=True,
                                stop=True,
                            )
                            if jc == ic:
                                off = (ib % 4) * P
                                nc.vector.tensor_tensor(
                                    pssc[:, off : off + P],
                                    pssc[:, off : off + P],
                                    tri[:],
                                    OP.add,
                                )
                            u = upool.tile([P, 512], F32R, tag="u")
                            nc.scalar.activation(
                                out=u[:, :w],
                                in_=pssc[:, :w],
                                func=AF.Exp,
                                accum_out=racc[:, jc : jc + 1],
                            )
                            u_chunks.append((u, w))
                        rinv = rpool.tile([P, 1], F32, tag="rinv")
                        if nch > 1:
                            rs = rpool.tile([P, 1], F32, tag="rsum")
                            nc.vector.reduce_sum(
                                rs[:], racc[:, :nch], axis=mybir.AxisListType.X
                            )
                            nc.vector.reciprocal(rinv[:], rs[:])
                        else:
                            nc.vector.reciprocal(rinv[:], racc[:, 0:1])
                        # normalize + DMA out + transpose into stage tiles
                        for jc in range(nch):
                            u, w = u_chunks[jc]
                            a = apool.tile([P, 512], F32, tag="att")
                            nc.gpsimd.tensor_scalar_mul(
                                a[:, :w], u[:, :w].bitcast(F32), rinv[:]
                            )
                            nc.sync.dma_start(
                                out=att_out.ap()[
                                    h, ib * P : (ib + 1) * P, jc * 512 : jc * 512 + w
                                ],
                                in_=a[:, :w],
                            )
                            for t in range(w // P):
                                js = jc * 4 + t
                                if js not in stages:
                                    stages[js] = stagep.tile([P, 512], F32R, tag="stg", name=f"stg{js}")
                                ptt = ps_t.tile([P, P], F32, tag="pst")
                                nc.tensor.transpose(
                                    ptt[:], a[:, t * P : (t + 1) * P], identf[:]
                                )
                                nc.vector.tensor_copy(
                                    stages[js][:, (ib % 4) * P : (ib % 4 + 1) * P],
                                    ptt[:],
                                )
                        if ib % 4 == 3:
                            # V-matmul for chunk ic over all j sub-blocks.
                            # PSUM dst must start at partition 0; odd heads are
                            # routed to attOT partitions 64:128 via SBUF DMA.
                            po = ps_o.tile([P, 512], F32, tag="po")
                            njs = 4 * ic + 4
                            for js in range(njs):
                                start_col = max(0, js - 4 * ic) * P
                                nc.tensor.matmul(
                                    po[0:DK, start_col:512],
                                    vS[:, js, h * DK : (h + 1) * DK],
                                    stages[js][:, start_col:512],
                                    start=(js == 0),
                                    stop=(js == njs - 1),
                                )
                            if hs == 0:
                                nc.vector.tensor_copy(
                                    attOT[0:DK, hp, ic * 512 : (ic + 1) * 512],
                                    po[0:DK, :],
                                )
                            else:
                                otmp = rpool.tile([P, 512], F32R, tag="otmp")
                                nc.vector.tensor_copy(otmp[0:DK, :], po[0:DK, :])
                                nc.sync.dma_start(
                                    out=attOT[DK:P, hp, ic * 512 : (ic + 1) * 512],
                                    in_=otmp[0:DK, :],
                                )
                            stages = {}

            # ---- Phase 4: output projection + reduce-scatter + layernorm ----
            cc_in = drampool.tile([L, DX], F32)
            cc_out = drampool.tile([HALF, DX], F32)
            with (
                tc.tile_pool(name="wo_p", bufs=1) as wo_p,
                tc.tile_pool(name="fc", bufs=3) as fcp,
                tc.tile_pool(name="qres", bufs=2) as qres,
                tc.tile_pool(name="ps_f", bufs=3, space="PSUM") as ps_f,
            ):
                wo_sb = wo_p.tile([P, DS, DX], F32R)
                nc.sync.dma_start(
                    out=wo_sb[:], in_=wo.ap().rearrange("(ho p) x -> p ho x", p=P)
                )
                bo_bc = wo_p.tile([P, DX], F32)
                nc.sync.dma_start(
                    out=bo_bc[:],
                    in_=bass.AP(tensor=bo_d.ap().tensor, offset=0, ap=[[0, P], [1, DX]]),
                )
                nc.gpsimd.tensor_scalar_mul(bo_bc[:], bo_bc[:], 0.5)
                for lb in range(LB):
                    qn = qres.tile([P, DX], F32, tag="qn")
                    nc.sync.dma_start(
                        out=qn[:],
                        in_=q_in.ap().bitcast(F32)[lb * P : (lb + 1) * P, :],
                    )
                    yp = fcp.tile([P, DX], F32, tag="yp")
                    for xc in range(2):
                        pf = ps_f.tile([P, 512], F32, tag="pf")
                        for hp in range(DS):
                            nc.tensor.matmul(
                                pf[:],
                                attOT[:, hp, lb * P : (lb + 1) * P],
                                wo_sb[:, hp, xc * 512 : (xc + 1) * 512],
                                start=(hp == 0),
                                stop=(hp == DS - 1),
                            )
                        # yp = 0.5*q_resid + fc_partial
                        nc.vector.scalar_tensor_tensor(
                            out=yp[:, xc * 512 : (xc + 1) * 512],
                            in0=qn[:, xc * 512 : (xc + 1) * 512],
                            scalar=0.5,
                            in1=pf[:],
                            op0=OP.mult,
                            op1=OP.add,
                        )
                    nc.gpsimd.tensor_tensor(yp[:], yp[:], bo_bc[:], OP.add)
                    nc.sync.dma_start(
                        out=cc_in[lb * P : (lb + 1) * P, :], in_=yp[:]
                    )
                nc.gpsimd.collective_compute(
                    "ReduceScatter",
                    OP.add,
                    replica_groups=RG,
                    ins=[cc_in[:].opt()],
                    outs=[cc_out[:].opt()],
                )

            with (
                tc.tile_pool(name="lnc", bufs=1) as lnc,
                tc.tile_pool(name="ln", bufs=3) as lnp,
            ):
                gam_bc = lnc.tile([P, DX], F32)
                nc.sync.dma_start(
                    out=gam_bc[:],
                    in_=bass.AP(tensor=gam_d.ap().tensor, offset=0, ap=[[0, P], [1, DX]]),
                )
                bet_bc = lnc.tile([P, DX], F32)
                nc.sync.dma_start(
                    out=bet_bc[:],
                    in_=bass.AP(tensor=bet_d.ap().tensor, offset=0, ap=[[0, P], [1, DX]]),
                )
                eps_t = lnc.tile([P, 1], F32)
                nc.vector.memset(eps_t[:], EPS)
                BNS = 512  # bn_stats max free dim
                for lb in range(HALF // P):
                    xt = lnp.tile([P, DX], F32, tag="x")
                    nc.sync.dma_start(out=xt[:], in_=cc_out[lb * P : (lb + 1) * P, :])
                    stats = lnp.tile([P, 2, 6], F32, tag="stats")
                    for sg in range(2):
                        nc.vector.bn_stats(
                            out=stats[:, sg, :], in_=xt[:, sg * BNS : (sg + 1) * BNS]
                        )
                    mv = lnp.tile([P, 2], F32, tag="mv")
                    nc.vector.bn_aggr(out=mv[:], in_=stats[:])
                    rstd = lnp.tile([P, 1], F32, tag="rstd")
                    nc.scalar.activation(
                        out=rstd[:], in_=mv[:, 1:2], func=AF.Sqrt,
                        bias=eps_t[:], scale=1.0,
                    )
                    nc.vector.reciprocal(rstd[:], rstd[:])
                    nc.vector.tensor_scalar(
                        xt[:], xt[:], mv[:, 0:1], rstd[:], op0=OP.subtract, op1=OP.mult
                    )
                    nc.vector.tensor_tensor(xt[:], xt[:], gam_bc[:], OP.mult)
                    nc.vector.tensor_tensor(xt[:], xt[:], bet_bc[:], OP.add)
                    nc.sync.dma_start(
                        out=y_out.ap()[lb * P : (lb + 1) * P, :], in_=xt[:]
                    )
    nc.compile()
    return nc


_NC_CACHE = None


def _get_nc():
    global _NC_CACHE
    if _NC_CACHE is None:
        nc = bacc.Bacc(
            "TRN2", target_bir_lowering=False, debug=False, num_devices=N_CORES
        )
        _NC_CACHE = build(nc)
    return _NC_CACHE


def kernel(Q, K, V, mask, Wq, bq, Wk, bk, Wv, bv, Wo, bo, gamma, beta, _trace=False):
    Q = np.ascontiguousarray(np.asarray(Q, dtype=np.float32))
    K = np.ascontiguousarray(np.asarray(K, dtype=np.float32))
    V = np.ascontiguousarray(np.asarray(V, dtype=np.float32))
    Wq = np.asarray(Wq, dtype=np.float32)
    Wk = np.asarray(Wk, dtype=np.float32)
    Wv = np.asarray(Wv, dtype=np.float32)
    Wo = np.asarray(Wo, dtype=np.float32)
    bq = np.asarray(bq, dtype=np.float32)
    bk = np.asarray(bk, dtype=np.float32)
    bv = np.asarray(bv, dtype=np.float32)
    bo = np.ascontiguousarray(np.asarray(bo, dtype=np.float32))
    gamma = np.ascontiguousarray(np.asarray(gamma, dtype=np.float32))
    beta = np.ascontiguousarray(np.asarray(beta, dtype=np.float32))

    nc = _get_nc()
    in_maps = []
    for c in range(N_CORES):
        b, g = c // 2, c % 2
        sl = slice(g * DHL, (g + 1) * DHL)
        in_maps.append(
            {
                "q_in": np.ascontiguousarray(Q[b]),
                "k_in": np.ascontiguousarray(K[b]),
                "v_in": np.ascontiguousarray(V[b]),
                "wq": np.ascontiguousarray(Wq[:, sl]),
                "wk": np.ascontiguousarray(Wk[:, sl]),
                "wv": np.ascontiguousarray(Wv[:, sl]),
                "wo": np.ascontiguousarray(Wo[sl, :]),
                "bq": np.ascontiguousarray(bq[sl]),
                "bk": np.ascontiguousarray(bk[sl]),
                "bv": np.ascontiguousarray(bv[sl]),
                "bo": bo,
                "gamma": gamma,
                "beta": beta,
            }
        )
    res = run_bass_kernel_spmd(
        nc, in_maps, core_ids=list(range(N_CORES)), trace=_trace
    )
    y = np.empty((B, L, DX), np.float32)
    att = np.zeros((H * B, L, L), np.float32)
    for c in range(N_CORES):
        b, g = c // 2, c % 2
        y[b, g * HALF : (g + 1) * HALF] = res.results[c]["y_out"]
        ao = res.results[c]["att_out"]
        for hl in range(HL):
            att[(g * HL + hl) * B + b] = ao[hl]
    kernel.last_results = res
    return (y, att)
